# revision 66
# baseline (speedup 1.0000x reference)
"""Multi-head attention Trainium2 kernel (B=8, N=1024, C=768, H=12, d=64).

Sharding: data-parallel over batch -- core b computes batch element b.

Per-core dataflow (fp16 matmul operands, fp32 PSUM accumulation):
  - host pre-transposes x -> xT [C, N], packs wq/wk block-major per head
    pair (one contiguous DMA each), folds the 1/sqrt(d) softmax scale into
    q_w, extends v_w with a zero column per head (slot for the
    softmax-denominator ones trick).
  - Qt = wqT.T @ xT [C, N], Kt likewise (heads on partitions);
    V' = xT.T @ vwT' [N, H*65] with col h*65+64 memset to 1.0.
  - per head-pair t, n-chunk, m-tile: St[m, n] = Kt_h.T @ Qt_h via the two
    64-row PE quadrants into one 2-bank PSUM tile; ONE 1024-wide exp on ACT
    covers both heads (halves ACT instruction count vs per-head exps).
  - AV accumulation runs one m-iteration behind the score stream (software
    pipeline) so the PE never stalls on the exp latency.  All projection
    work that is not needed yet (Q/K for later pairs, V' half for pairs
    3-5, the output projection for early token tiles) is emitted as
    small filler steps paced into the attention stream, which keeps the
    PE busy enough that the HAM clock gate never throttles it to 1.2 GHz.
  - PSUM: 4 banks score double-buffer, 2 ya accumulators, 2 filler banks;
    pool alternation keeps PE->DVE eviction latency off the critical path.
  - softmax denominators: reciprocal_approx_fast on a partition-0 staging
    row (HW quirks: partition_broadcast needs partition-0 input and
    offset-0 full-height output; recip_approx needs SBUF offset-0 input).
  - Yt[hd, n] = yt * head_mask[h]^2 / colsum; out = Yt.T @ pwT, fp16.
  - junk matmuls during the initial DMA wait pre-warm the HAM clock gate;
    a dummy exp preloads the ACT table.
"""

import numpy as np

B, N, C, H, D = 8, 1024, 768, 12, 64
KO = C // 128          # 6 contraction tiles of 128 channels
MT = N // 128          # 8 token tiles
NCH = N // 512         # 2 free-dim chunks of 512
D1 = D + 1             # V' block width per head (64 V cols + 1 ones col)
CV = H * D1            # 780 extended V channels
NCORES = 8

MM_DTYPE = "f16"

_cache = {}


def _build():
    import concourse.bacc as bacc
    import concourse.mybir as mybir
    import concourse.tile as tile
    from collections import deque

    F32 = mybir.dt.float32
    F16 = mybir.dt.float16
    MMD = {"bf16": mybir.dt.bfloat16, "f16": mybir.dt.float16,
           "f32r": mybir.dt.float32r, "f32": mybir.dt.float32}[MM_DTYPE]
    AF = mybir.ActivationFunctionType

    nc = bacc.Bacc("TRN2", target_bir_lowering=False, debug=False)

    d_xT = nc.dram_tensor("xT", [C, N], MMD, kind="ExternalInput")
    # wq/wk host-packed block-major: [t, p, ko*128] so one pair block is a
    # single contiguous DMA (1.5KB/partition descriptors)
    d_wq = nc.dram_tensor("wqT", [KO, 128, C], MMD, kind="ExternalInput")
    d_wk = nc.dram_tensor("wkT", [KO, 128, C], MMD, kind="ExternalInput")
    d_wv = nc.dram_tensor("vwT", [C, CV], MMD, kind="ExternalInput")
    d_wp = nc.dram_tensor("pwT", [C, C], MMD, kind="ExternalInput")
    d_out = nc.dram_tensor("out", [N, C], F16, kind="ExternalOutput")

    r_xT = d_xT.ap().rearrange("(ko p) n -> p ko n", p=128)
    r_wq = d_wq.ap().rearrange("t p (ko j) -> t p ko j", j=128)
    r_wk = d_wk.ap().rearrange("t p (ko j) -> t p ko j", j=128)
    r_wv = d_wv.ap().rearrange("(ko p) m -> p ko m", p=128)
    r_wp = d_wp.ap().rearrange("(ko p) m -> p ko m", p=128)
    r_out = d_out.ap().rearrange("(mt p) c -> mt p c", p=128)

    with tile.TileContext(nc) as tc:
        with (
            tc.tile_pool(name="xw", bufs=1) as xw,          # resident tensors
            tc.tile_pool(name="wq", bufs=3) as wqp,         # streamed weight blocks
            tc.tile_pool(name="wk", bufs=3) as wkp,
            tc.tile_pool(name="qt", bufs=3) as qtp,         # Qt/Kt streamed per pair
            tc.tile_pool(name="kt", bufs=3) as ktp,
            tc.tile_pool(name="vp", bufs=8) as vpp,         # V' all 8 token tiles
            tc.tile_pool(name="yt", bufs=6) as ytp,         # Yt all 6 channel tiles
            tc.tile_pool(name="pp", bufs=3) as ppp,         # P = exp(St), [128,2,512]
            tc.tile_pool(name="cs", bufs=2) as csp,         # colsum / recip rows
            tc.tile_pool(name="bc", bufs=2) as bcp,         # broadcast recip tiles
            tc.tile_pool(name="ob", bufs=2) as obp,         # output staging
            tc.tile_pool(name="st", bufs=2, space="PSUM") as stp,   # 4 banks
            tc.tile_pool(name="ya", bufs=2, space="PSUM") as yap,   # 2 banks
            tc.tile_pool(name="mm", bufs=2, space="PSUM") as mmp,   # 2 banks
        ):
            # ---- warmup: preload ACT exp table + keep PE HAM gate warm
            # while input DMAs land (nothing below depends on these) ----
            t_junk = xw.tile([128, 640], MMD, tag="junk")
            nc.vector.memset(t_junk[:], 0.01)
            t_j32 = xw.tile([128, 16], F32, tag="j32")
            nc.vector.memset(t_j32[:], 0.0)
            t_je = xw.tile([128, 16], MMD, tag="je")
            nc.scalar.activation(t_je[:], t_j32[:], AF.Exp)

            # ---- input DMAs in priority order ----
            def qk_dma(t):
                t_wqb = wqp.tile([128, KO, 128], MMD, tag="wq", name=f"wqb{t}")
                t_wkb = wkp.tile([128, KO, 128], MMD, tag="wk", name=f"wkb{t}")
                nc.sync.dma_start(out=t_wqb[:], in_=r_wq[t])
                nc.sync.dma_start(out=t_wkb[:], in_=r_wk[t])
                return t_wqb, t_wkb

            wb = {0: qk_dma(0)}

            t_x = xw.tile([128, KO, N], MMD, tag="x")
            t_wv = xw.tile([128, KO, CV], MMD, tag="wv")
            for ko in range(KO):
                nc.sync.dma_start(out=t_x[:, ko, 0:512], in_=r_xT[:, ko, 0:512])
            for ko in range(KO):
                nc.sync.dma_start(out=t_x[:, ko, 512:N], in_=r_xT[:, ko, 512:N])
            for ko in range(KO):
                nc.sync.dma_start(
                    out=t_wv[:, ko, 0:390], in_=r_wv[:, ko, 0:390]
                )
            for ko in range(KO):
                nc.sync.dma_start(
                    out=t_wv[:, ko, 390:CV], in_=r_wv[:, ko, 390:CV]
                )

            # junk matmuls warm the HAM clock gate while input DMAs land, so
            # the first real matmuls run at full clock; rotating stp slots
            # keeps them dense (no single-slot WAW serialization)
            for i in range(10):
                ps = stp.tile([128, 2, 512], F32, tag="st", name="warm")
                nc.tensor.matmul(
                    ps[:, 0, :], t_junk[:, 0:128], t_junk[:, 128:640],
                    start=True, stop=True,
                )

            # ---- Q/K projection units (one PSUM accumulation group each),
            # exposed as single-matmul steps for filler pacing ----
            def qk_tiles(t):
                t_q = qtp.tile([128, N], MMD, tag="qt", name=f"q{t}")
                t_k = ktp.tile([128, N], MMD, tag="kt", name=f"k{t}")
                return t_q, t_k

            def st_bank():
                return stp.tile([128, 2, 512], F32, tag="st", name="stb")[:, 0, :]

            def mm_bank():
                return mmp.tile([128, 512], F32, tag="mm", name="mmb")

            def qk_unit_steps(t, t_q, t_k, pools=(mm_bank,)):
                """Yield once per projection matmul for pair t (24 total)."""
                t_wqb, t_wkb = wb[t]
                for i, (wsrc, dst, ch) in enumerate((
                    (t_wqb, t_q, 0), (t_wkb, t_k, 0),
                    (t_wkb, t_k, 1), (t_wqb, t_q, 1),
                )):
                    nsl = slice(ch * 512, (ch + 1) * 512)
                    ps = pools[i % len(pools)]()
                    for ko in range(KO):
                        nc.tensor.matmul(
                            ps[:], wsrc[:, ko, :], t_x[:, ko, nsl],
                            start=(ko == 0), stop=(ko == KO - 1),
                        )
                        if ko < KO - 1:
                            yield
                    nc.vector.tensor_copy(dst[:, nsl], ps[:])
                    yield

            # ---- V' projection for one token tile: V'[n, cv] = xT.T @ vwT.
            # Half 0 covers head pairs 0-2 (cols 0:390), half 1 pairs 3-5;
            # one PSUM group per half, yielding mid-group and at the end ----
            t_v = [None] * MT

            def vprime_steps(mt, half):
                if half == 0:
                    t_v[mt] = vpp.tile([128, CV], MMD, tag="v", name=f"v{mt}")
                tv = t_v[mt]
                c0 = half * 390
                ps = mmp.tile([128, 512], F32, tag="mm", name=f"pv{mt}{half}")
                for ko in range(KO):
                    nc.tensor.matmul(
                        ps[:, 0:390],
                        t_x[:, ko, mt * 128:(mt + 1) * 128],
                        t_wv[:, ko, c0:c0 + 390],
                        start=(ko == 0), stop=(ko == KO - 1),
                    )
                    if ko == 2:
                        yield
                nc.vector.tensor_copy(tv[:, c0:c0 + 390], ps[:, 0:390])
                ones_cols = tv[:, c0:c0 + 390].rearrange(
                    "p (h e) -> p h e", e=D1
                )[:, :, D:D + 1]
                nc.vector.memset(ones_cols, 1.0)
                yield

            # ---- startup: QK0 units + V'[0] half 0 before the attention
            # loop; alternate PSUM pools so unit N+1 never waits on unit N's
            # eviction (stp is free until the attention stream begins) ----
            q0, k0 = qk_tiles(0)
            g0 = qk_unit_steps(0, q0, k0, pools=(mm_bank, st_bank))
            for _ in range(18):   # all of k0 + q0's ch0 half
                next(g0)


            def vhalf1_gen():
                for mt in range(MT):
                    yield from vprime_steps(mt, 1)

            # ---- output projection steps: out[n, c] = Yt.T @ pwT, one
            # mm-bank group per (token tile, column chunk) ----
            wpb = {}

            def outproj_steps(mts, pools=None):
                for mt in mts:
                    msl = slice(mt * 128, (mt + 1) * 128)
                    t_o = obp.tile([128, C], MMD, tag="ob", name=f"ob{mt}")
                    for ci, (c0, cw) in enumerate(((0, 512), (512, 256))):
                        ps = pools[ci]() if pools else mmp.tile(
                            [128, 512], F32, tag="mm", name=f"po{mt}{c0}"
                        )
                        for tt in range(KO):
                            nc.tensor.matmul(
                                ps[:, :cw], t_yt[tt][:, msl],
                                wpb["wp"][:, tt, c0:c0 + cw],
                                start=(tt == 0), stop=(tt == KO - 1),
                            )
                            if tt < KO - 1:
                                yield
                        nc.vector.tensor_copy(t_o[:, c0:c0 + cw], ps[:, :cw])
                        nc.sync.dma_start(
                            out=r_out[mt, :, c0:c0 + cw], in_=t_o[:, c0:c0 + cw]
                        )
                        yield

            # partial chunk-A groups for early token tiles: accumulate
            # pairs 0..4 while pair 5 is still streaming, finish with the
            # tt=5 matmul once Yt[5] is normalized
            opart = {}

            def outproj_partial(mts):
                for mt in mts:
                    msl = slice(mt * 128, (mt + 1) * 128)
                    ps = mm_bank()
                    for tt in range(KO - 1):
                        nc.tensor.matmul(
                            ps[:], t_yt[tt][:, msl], wpb["wp"][:, tt, 0:512],
                            start=(tt == 0), stop=False,
                        )
                        yield
                    opart[mt] = ps

            def outproj_finish(mts):
                for mt in mts:
                    msl = slice(mt * 128, (mt + 1) * 128)
                    ps = opart.pop(mt)
                    t_o = obp.tile([128, C], MMD, tag="ob", name=f"obw{mt}")
                    nc.tensor.matmul(
                        ps[:], t_yt[KO - 1][:, msl],
                        wpb["wp"][:, KO - 1, 0:512], start=False, stop=True,
                    )
                    yield
                    nc.vector.tensor_copy(t_o[:, 0:512], ps[:])
                    nc.sync.dma_start(
                        out=r_out[mt, :, 0:512], in_=t_o[:, 0:512]
                    )
                    psB = mm_bank()
                    for tt in range(KO):
                        nc.tensor.matmul(
                            psB[:, 0:256], t_yt[tt][:, msl],
                            wpb["wp"][:, tt, 512:C],
                            start=(tt == 0), stop=(tt == KO - 1),
                        )
                        if tt < KO - 1:
                            yield
                    nc.vector.tensor_copy(t_o[:, 512:C], psB[:, 0:256])
                    nc.sync.dma_start(
                        out=r_out[mt, :, 512:C], in_=t_o[:, 512:C]
                    )
                    yield

            # ---- filler queue: generators of single-matmul steps ----
            filler_q = deque()   # (t_needed_by, generator)
            filler_q.append((1, g0))

            def fill(n):
                did = 0
                while n > 0 and filler_q:
                    try:
                        next(filler_q[0][1])
                        n -= 1
                        did += 1
                    except StopIteration:
                        filler_q.popleft()
                return did

            def drain_through(t_limit):
                keep = []
                while filler_q:
                    tnb, g = filler_q.popleft()
                    if tnb <= t_limit:
                        for _ in g:
                            pass
                    else:
                        keep.append((tnb, g))
                filler_q.extend(keep)

            # ---- the fused score/exp/AV pipeline ----
            t_yt = [None] * KO
            ya = {}      # (t, ch) -> (ya0, ya1)
            qk = {0: (q0, k0)}
            pipe = deque()  # (t, ch, mt, p_tile), AV runs two iters behind

            def emit_av(s):
                pt, pch, pmt, pp_ = s
                nsl = slice(pch * 512, (pch + 1) * 512)
                if pmt == 0:
                    ya[(pt, pch)] = (
                        yap.tile([D1, 512], F32, tag="ya", name=f"ya{pt}{pch}0"),
                        yap.tile([D1, 512], F32, tag="ya", name=f"ya{pt}{pch}1"),
                    )
                ya0, ya1 = ya[(pt, pch)]
                nc.tensor.matmul(
                    ya0[:], t_v[pmt][:, (2 * pt) * D1:(2 * pt + 1) * D1],
                    pp_[:, 0, :], start=(pmt == 0), stop=(pmt == MT - 1),
                )
                nc.tensor.matmul(
                    ya1[:], t_v[pmt][:, (2 * pt + 1) * D1:(2 * pt + 2) * D1],
                    pp_[:, 1, :], start=(pmt == 0), stop=(pmt == MT - 1),
                )
                if pmt == MT - 1:
                    # evict unnormalized yt; reciprocal of each colsum row
                    # straight off PSUM into a partition-0 staging row
                    # (partition_broadcast needs partition-0 input and
                    # offset-0 full-height output on HW)
                    # reciprocal_approx_fast needs SBUF input at partition
                    # offset 0 on HW: stage both colsum rows first, then one
                    # batched approx-recip over [1, 2, 512]
                    t_ss = csp.tile([1, 2, 512], F32, tag="ss",
                                    name=f"ss{pt}{pch}")
                    t_rs = csp.tile([1, 2, 512], F32, tag="rs",
                                    name=f"rs{pt}{pch}")
                    nc.vector.tensor_copy(t_yt[pt][0:64, nsl], ya0[0:D, :])
                    nc.vector.tensor_copy(t_ss[0:1, 0, :], ya0[D:D1, :])
                    nc.vector.tensor_copy(t_yt[pt][64:128, nsl], ya1[0:D, :])
                    nc.vector.tensor_copy(t_ss[0:1, 1, :], ya1[D:D1, :])
                    nc.vector.reciprocal_approx_fast(t_rs[0:1], t_ss[0:1])
                    del ya[(pt, pch)]
                    for hp in range(2):
                        psl = slice(hp * 64, hp * 64 + 64)
                        t_bc = bcp.tile([128, 512], F32, tag="bc",
                                        name=f"bc{pt}{pch}{hp}")
                        nc.gpsimd.partition_broadcast(
                            t_bc[:], t_rs[0:1, hp, :]
                        )
                        nc.vector.tensor_mul(
                            t_yt[pt][psl, nsl], t_yt[pt][psl, nsl],
                            t_bc[psl, :]
                        )

            for t in range(KO):
                t_yt[t] = ytp.tile([128, N], MMD, tag="yt", name=f"yt{t}")
                if t + 1 < KO:
                    wb[t + 1] = qk_dma(t + 1)
                    tq1, tk1 = qk_tiles(t + 1)
                    qk[t + 1] = (tq1, tk1)
                    filler_q.append((t + 1, qk_unit_steps(t + 1, tq1, tk1)))
                if t == 1:
                    # V' half 1 (pairs 3-5): paced behind qk(2), due by t=3
                    filler_q.append((3, vhalf1_gen()))
                drain_through(t)  # safety: pair t fully projected
                if t == 1:
                    t_wp = xw.tile([128, KO, C], MMD, tag="wpf")
                    for ko in range(KO):
                        nc.sync.dma_start(out=t_wp[:, ko, :], in_=r_wp[:, ko, :])
                    wpb["wp"] = t_wp
                t_q, t_k = qk[t]
                for ch in range(NCH):
                    nsl = slice(ch * 512, (ch + 1) * 512)
                    for mt in range(MT):
                        msl = slice(mt * 128, (mt + 1) * 128)
                        st2 = stp.tile([128, 2, 512], F32, tag="st",
                                       name=f"st{t}{ch}{mt}")
                        nc.tensor.matmul(
                            st2[:, 0, :], t_k[0:64, msl], t_q[0:64, nsl],
                            start=True, stop=True, tile_position=(0, 0),
                        )
                        nc.tensor.matmul(
                            st2[:, 1, :], t_k[64:128, msl], t_q[64:128, nsl],
                            start=True, stop=True, tile_position=(64, 0),
                        )
                        p = ppp.tile([128, 2, 512], MMD, tag="p",
                                     name=f"p{t}{ch}{mt}")
                        nc.scalar.activation(p[:], st2[:], AF.Exp)
                        vg = None
                        if t == 0 and ch == 0:
                            # weave V'[mt] half-0 just in time for its AV
                            # (next iteration); the group tail lands after
                            # the AV pair so its eviction wait stays off
                            # the critical path
                            vg = vprime_steps(mt, 0)
                            next(vg)
                            fill(2)   # q0's ch1 half
                        elif t == 0:
                            fill(3)   # QK(1) units: 24 steps over 8 iters
                        else:
                            if t == KO - 1 and ch == 0 and mt == 0:
                                filler_q.append(
                                    (KO, outproj_partial(range(0, 2)))
                                )
                            if t == KO - 1 and ch == 1 and mt == 1:
                                # Yt chunk 0 of every pair is normalized by
                                # now: weave the first half of the output
                                # projection into the final chunk's stream
                                filler_q.append(
                                    (KO, outproj_finish(range(0, 2)))
                                )
                                filler_q.append(
                                    (KO, outproj_steps(range(2, MT // 2)))
                                )
                            quota = 7 if (t == KO - 1 and ch == 1) else 2
                            fill(quota)
                        if len(pipe) >= 1:
                            emit_av(pipe.popleft())
                        pipe.append((t, ch, mt, p))
                        if vg is not None:
                            for _ in vg:
                                pass

            while pipe:  # flush the final AV pairs + eviction + norm
                emit_av(pipe.popleft())
            drain_through(KO)  # finish any leftover out-proj weave steps

            # tail: the 512-col groups for mt 4..7 accumulate pairs 0..4
            # (already normalized) while pair 5's normalization chain runs
            # on DVE/GpSimd, so the PE never idles on the final flush
            tailA = []
            for i, mt in enumerate(range(MT // 2, MT)):
                msl = slice(mt * 128, (mt + 1) * 128)
                ps = (st_bank if i % 2 == 0 else mm_bank)()
                for tt in range(KO - 1):
                    nc.tensor.matmul(
                        ps[:], t_yt[tt][:, msl], wpb["wp"][:, tt, 0:512],
                        start=(tt == 0), stop=False,
                    )
                tailA.append(ps)
            for i, mt in enumerate(range(MT // 2, MT)):
                msl = slice(mt * 128, (mt + 1) * 128)
                t_o = obp.tile([128, C], MMD, tag="ob", name=f"obt{mt}")
                nc.tensor.matmul(
                    tailA[i], t_yt[KO - 1][:, msl],
                    wpb["wp"][:, KO - 1, 0:512], start=False, stop=True,
                )
                nc.vector.tensor_copy(t_o[:, 0:512], tailA[i])
                nc.sync.dma_start(out=r_out[mt, :, 0:512], in_=t_o[:, 0:512])
                psB = (mm_bank if i % 2 == 0 else st_bank)()
                for tt in range(KO):
                    nc.tensor.matmul(
                        psB[:, 0:256], t_yt[tt][:, msl],
                        wpb["wp"][:, tt, 512:C],
                        start=(tt == 0), stop=(tt == KO - 1),
                    )
                nc.vector.tensor_copy(t_o[:, 512:C], psB[:, 0:256])
                nc.sync.dma_start(out=r_out[mt, :, 512:C], in_=t_o[:, 512:C])

    nc.compile()
    return nc


def _prep_inputs(x, head_mask, q_w, k_w, v_w, proj_w):
    import ml_dtypes

    mmnp = {"bf16": ml_dtypes.bfloat16, "f16": np.float16,
            "f32r": np.float32, "f32": np.float32}[MM_DTYPE]
    def pack_blocks(wT):
        # [C_in, C_out] -> [t, p, ko*128+j] with wT[ko*128+p, t*128+j]
        return np.ascontiguousarray(
            wT.reshape(KO, 128, KO, 128).transpose(2, 1, 0, 3).reshape(KO, 128, C)
        )

    scale = np.float32(D ** -0.5)
    wqT = pack_blocks((q_w.T * scale).astype(np.float32)).astype(mmnp)
    wkT = pack_blocks(k_w.T.astype(np.float32)).astype(mmnp)
    vwT0 = np.zeros((C, CV), np.float32)
    vT = v_w.T.astype(np.float32)
    for h in range(H):
        vwT0[:, h * D1:h * D1 + D] = vT[:, h * D:(h + 1) * D]
    pwT = np.ascontiguousarray(proj_w.T).astype(mmnp)
    in_maps = []
    for b in range(NCORES):
        xT = np.ascontiguousarray(x[b].T).astype(mmnp)
        # fold head_mask^2 into this core's V weights (ones cols stay 0->1)
        vwT = vwT0.copy()
        for h in range(H):
            vwT[:, h * D1:h * D1 + D] *= head_mask[b, h] ** 2
        in_maps.append(
            {"xT": xT, "wqT": wqT, "wkT": wkT, "vwT": vwT.astype(mmnp),
             "pwT": pwT}
        )
    return in_maps


def _run(inputs, trace=False):
    from concourse.bass_utils import run_bass_kernel_spmd

    x = np.asarray(inputs["x"], np.float32)
    head_mask = np.asarray(inputs["head_mask"], np.float32)
    in_maps = _prep_inputs(
        x,
        head_mask,
        np.asarray(inputs["q_w"], np.float32),
        np.asarray(inputs["k_w"], np.float32),
        np.asarray(inputs["v_w"], np.float32),
        np.asarray(inputs["proj_w"], np.float32),
    )
    # biases are zero by construction of this problem (spec fill=zeros);
    # q_b/k_b/v_b/proj_b are validated and otherwise unused.
    for name in ("q_b", "k_b", "v_b", "proj_b"):
        bias = np.asarray(inputs[name])
        if np.abs(bias).max() > 0:
            raise NotImplementedError(f"nonzero {name} not supported")

    if "nc" not in _cache:
        _cache["nc"] = _build()
    nc = _cache["nc"]
    res = run_bass_kernel_spmd(
        nc, in_maps, core_ids=list(range(NCORES)), trace=trace
    )
    out = np.stack([res.results[b]["out"] for b in range(NCORES)], axis=0)
    return out.astype(np.float32), res


def kernel(**inputs):
    out, _ = _run(inputs, trace=False)
    return out


# revision 67
# speedup vs baseline: 1.1769x; 1.1769x over previous
"""Multi-head attention Trainium2 kernel (B=8, N=1024, C=768, H=12, d=64).

Sharding: data-parallel over batch -- core b computes batch element b.

Per-core dataflow (fp16 matmul operands, fp32 PSUM accumulation):
  - host pre-transposes x -> xT [C, N], packs wq/wk block-major per head
    pair (one contiguous DMA each), folds the 1/sqrt(d) softmax scale into
    q_w, extends v_w with a zero column per head (slot for the
    softmax-denominator ones trick).
  - Qt = wqT.T @ xT [C, N], Kt likewise (heads on partitions);
    V' = xT.T @ vwT' [N, H*65] with col h*65+64 memset to 1.0.
  - per head-pair t, n-chunk, m-tile: St[m, n] = Kt_h.T @ Qt_h via the two
    64-row PE quadrants into one 2-bank PSUM tile; ONE 1024-wide exp on ACT
    covers both heads (halves ACT instruction count vs per-head exps).
  - AV accumulation runs one m-iteration behind the score stream (software
    pipeline) so the PE never stalls on the exp latency.  All projection
    work that is not needed yet (Q/K for later pairs, V' half for pairs
    3-5, the output projection for early token tiles) is emitted as
    small filler steps paced into the attention stream, which keeps the
    PE busy enough that the HAM clock gate never throttles it to 1.2 GHz.
  - PSUM: 4 banks score double-buffer, 2 ya accumulators, 2 filler banks;
    pool alternation keeps PE->DVE eviction latency off the critical path.
  - softmax denominators: reciprocal_approx_fast on a partition-0 staging
    row (HW quirks: partition_broadcast needs partition-0 input and
    offset-0 full-height output; recip_approx needs SBUF offset-0 input).
  - Yt[hd, n] = yt * head_mask[h]^2 / colsum; out = Yt.T @ pwT, fp16.
  - junk matmuls during the initial DMA wait pre-warm the HAM clock gate;
    a dummy exp preloads the ACT table.
"""

import numpy as np

B, N, C, H, D = 8, 1024, 768, 12, 64
KO = C // 128          # 6 contraction tiles of 128 channels
MT = N // 128          # 8 token tiles
NCH = N // 512         # 2 free-dim chunks of 512
D1 = D + 1             # V' block width per head (64 V cols + 1 ones col)
CV = H * D1            # 780 extended V channels
NCORES = 8

MM_DTYPE = "f16"

_cache = {}


def _build():
    import concourse.bacc as bacc
    import concourse.mybir as mybir
    import concourse.tile as tile
    from collections import deque

    F32 = mybir.dt.float32
    F16 = mybir.dt.float16
    MMD = {"bf16": mybir.dt.bfloat16, "f16": mybir.dt.float16,
           "f32r": mybir.dt.float32r, "f32": mybir.dt.float32}[MM_DTYPE]
    AF = mybir.ActivationFunctionType

    nc = bacc.Bacc("TRN2", target_bir_lowering=False, debug=False)

    d_xT = nc.dram_tensor("xT", [C, N], MMD, kind="ExternalInput")
    # wq/wk host-packed block-major: [t, p, ko*128] so one pair block is a
    # single contiguous DMA (1.5KB/partition descriptors)
    d_wq = nc.dram_tensor("wqT", [KO, 128, C], MMD, kind="ExternalInput")
    d_wk = nc.dram_tensor("wkT", [KO, 128, C], MMD, kind="ExternalInput")
    d_wv = nc.dram_tensor("vwT", [C, CV], MMD, kind="ExternalInput")
    d_wp = nc.dram_tensor("pwT", [C, C], MMD, kind="ExternalInput")
    d_out = nc.dram_tensor("out", [N, C], F16, kind="ExternalOutput")

    r_xT = d_xT.ap().rearrange("(ko p) n -> p ko n", p=128)
    r_wq = d_wq.ap().rearrange("t p (ko j) -> t p ko j", j=128)
    r_wk = d_wk.ap().rearrange("t p (ko j) -> t p ko j", j=128)
    r_wv = d_wv.ap().rearrange("(ko p) m -> p ko m", p=128)
    r_wp = d_wp.ap().rearrange("(ko p) m -> p ko m", p=128)
    r_out = d_out.ap().rearrange("(mt p) c -> mt p c", p=128)

    with tile.TileContext(nc) as tc:
        with (
            tc.tile_pool(name="xw", bufs=1) as xw,          # resident tensors
            tc.tile_pool(name="wq", bufs=3) as wqp,         # streamed weight blocks
            tc.tile_pool(name="wk", bufs=3) as wkp,
            tc.tile_pool(name="qt", bufs=3) as qtp,         # Qt/Kt streamed per pair
            tc.tile_pool(name="kt", bufs=3) as ktp,
            tc.tile_pool(name="vp", bufs=8) as vpp,         # V' all 8 token tiles
            tc.tile_pool(name="yt", bufs=6) as ytp,         # Yt all 6 channel tiles
            tc.tile_pool(name="pp", bufs=3) as ppp,         # P = exp(St), [128,2,512]
            tc.tile_pool(name="cs", bufs=2) as csp,         # colsum / recip rows
            tc.tile_pool(name="bc", bufs=2) as bcp,         # broadcast recip tiles
            tc.tile_pool(name="ob", bufs=2) as obp,         # output staging
            tc.tile_pool(name="st", bufs=2, space="PSUM") as stp,   # 4 banks
            tc.tile_pool(name="ya", bufs=2, space="PSUM") as yap,   # 2 banks
            tc.tile_pool(name="mm", bufs=2, space="PSUM") as mmp,   # 2 banks
        ):
            # ---- warmup: preload ACT exp table + keep PE HAM gate warm
            # while input DMAs land (nothing below depends on these) ----
            t_junk = xw.tile([128, 640], MMD, tag="junk")
            nc.vector.memset(t_junk[:], 0.01)
            t_j32 = xw.tile([128, 16], F32, tag="j32")
            nc.vector.memset(t_j32[:], 0.0)
            t_je = xw.tile([128, 16], MMD, tag="je")
            nc.scalar.activation(t_je[:], t_j32[:], AF.Exp)

            # ---- input DMAs in priority order ----
            def qk_dma(t):
                t_wqb = wqp.tile([128, KO, 128], MMD, tag="wq", name=f"wqb{t}")
                t_wkb = wkp.tile([128, KO, 128], MMD, tag="wk", name=f"wkb{t}")
                nc.sync.dma_start(out=t_wqb[:], in_=r_wq[t])
                nc.sync.dma_start(out=t_wkb[:], in_=r_wk[t])
                return t_wqb, t_wkb

            wb = {0: qk_dma(0)}

            t_x = xw.tile([128, KO, N], MMD, tag="x")
            t_wv = xw.tile([128, KO, CV], MMD, tag="wv")
            for ko in range(KO):
                nc.sync.dma_start(out=t_x[:, ko, 0:512], in_=r_xT[:, ko, 0:512])
            for ko in range(KO):
                nc.sync.dma_start(out=t_x[:, ko, 512:N], in_=r_xT[:, ko, 512:N])
            for ko in range(KO):
                nc.sync.dma_start(
                    out=t_wv[:, ko, 0:390], in_=r_wv[:, ko, 0:390]
                )
            for ko in range(KO):
                nc.sync.dma_start(
                    out=t_wv[:, ko, 390:CV], in_=r_wv[:, ko, 390:CV]
                )

            # junk matmuls warm the HAM clock gate while input DMAs land, so
            # the first real matmuls run at full clock; rotating stp slots
            # keeps them dense (no single-slot WAW serialization)
            for i in range(10):
                ps = stp.tile([128, 2, 512], F32, tag="st", name="warm")
                nc.tensor.matmul(
                    ps[:, 0, :], t_junk[:, 0:128], t_junk[:, 128:640],
                    start=True, stop=True,
                )

            # ---- Q/K projection units (one PSUM accumulation group each),
            # exposed as single-matmul steps for filler pacing ----
            def qk_tiles(t):
                t_q = qtp.tile([128, N], MMD, tag="qt", name=f"q{t}")
                t_k = ktp.tile([128, N], MMD, tag="kt", name=f"k{t}")
                return t_q, t_k

            def st_bank():
                return stp.tile([128, 2, 512], F32, tag="st", name="stb")[:, 0, :]

            def mm_bank():
                return mmp.tile([128, 512], F32, tag="mm", name="mmb")

            def qk_unit_steps(t, t_q, t_k, pools=(mm_bank,)):
                """Yield once per projection matmul for pair t (24 total)."""
                t_wqb, t_wkb = wb[t]
                for i, (wsrc, dst, ch) in enumerate((
                    (t_wqb, t_q, 0), (t_wkb, t_k, 0),
                    (t_wkb, t_k, 1), (t_wqb, t_q, 1),
                )):
                    nsl = slice(ch * 512, (ch + 1) * 512)
                    ps = pools[i % len(pools)]()
                    for ko in range(KO):
                        nc.tensor.matmul(
                            ps[:], wsrc[:, ko, :], t_x[:, ko, nsl],
                            start=(ko == 0), stop=(ko == KO - 1),
                        )
                        if ko < KO - 1:
                            yield
                    nc.vector.tensor_copy(dst[:, nsl], ps[:])
                    yield

            # ---- V' projection for one token tile: V'[n, cv] = xT.T @ vwT.
            # Half 0 covers head pairs 0-2 (cols 0:390), half 1 pairs 3-5;
            # one PSUM group per half, yielding mid-group and at the end ----
            t_v = [None] * MT

            def vprime_steps(mt, half):
                if half == 0:
                    t_v[mt] = vpp.tile([128, CV], MMD, tag="v", name=f"v{mt}")
                tv = t_v[mt]
                c0 = half * 390
                ps = mmp.tile([128, 512], F32, tag="mm", name=f"pv{mt}{half}")
                for ko in range(KO):
                    nc.tensor.matmul(
                        ps[:, 0:390],
                        t_x[:, ko, mt * 128:(mt + 1) * 128],
                        t_wv[:, ko, c0:c0 + 390],
                        start=(ko == 0), stop=(ko == KO - 1),
                    )
                    if ko == 2:
                        yield
                nc.vector.tensor_copy(tv[:, c0:c0 + 390], ps[:, 0:390])
                ones_cols = tv[:, c0:c0 + 390].rearrange(
                    "p (h e) -> p h e", e=D1
                )[:, :, D:D + 1]
                nc.vector.memset(ones_cols, 1.0)
                yield

            # ---- startup: QK0 units + V'[0] half 0 before the attention
            # loop; alternate PSUM pools so unit N+1 never waits on unit N's
            # eviction (stp is free until the attention stream begins) ----
            q0, k0 = qk_tiles(0)
            g0 = qk_unit_steps(0, q0, k0, pools=(mm_bank, st_bank))
            for _ in range(18):   # all of k0 + q0's ch0 half
                next(g0)
            for _ in vprime_steps(0, 0):
                pass
            for _ in vprime_steps(1, 0):
                pass


            def vhalf1_gen():
                for mt in range(MT):
                    yield from vprime_steps(mt, 1)

            # ---- output projection steps: out[n, c] = Yt.T @ pwT, one
            # mm-bank group per (token tile, column chunk) ----
            wpb = {}

            def outproj_steps(mts, pools=None):
                for mt in mts:
                    msl = slice(mt * 128, (mt + 1) * 128)
                    t_o = obp.tile([128, C], MMD, tag="ob", name=f"ob{mt}")
                    for ci, (c0, cw) in enumerate(((0, 512), (512, 256))):
                        ps = pools[ci]() if pools else mmp.tile(
                            [128, 512], F32, tag="mm", name=f"po{mt}{c0}"
                        )
                        for tt in range(KO):
                            nc.tensor.matmul(
                                ps[:, :cw], t_yt[tt][:, msl],
                                wpb["wp"][:, tt, c0:c0 + cw],
                                start=(tt == 0), stop=(tt == KO - 1),
                            )
                            if tt < KO - 1:
                                yield
                        nc.vector.tensor_copy(t_o[:, c0:c0 + cw], ps[:, :cw])
                        nc.sync.dma_start(
                            out=r_out[mt, :, c0:c0 + cw], in_=t_o[:, c0:c0 + cw]
                        )
                        yield

            # partial chunk-A groups for early token tiles: accumulate
            # pairs 0..4 while pair 5 is still streaming, finish with the
            # tt=5 matmul once Yt[5] is normalized
            opart = {}

            def outproj_partial(mts):
                for mt in mts:
                    msl = slice(mt * 128, (mt + 1) * 128)
                    ps = mm_bank()
                    for tt in range(KO - 1):
                        nc.tensor.matmul(
                            ps[:], t_yt[tt][:, msl], wpb["wp"][:, tt, 0:512],
                            start=(tt == 0), stop=False,
                        )
                        yield
                    opart[mt] = ps

            def outproj_finish(mts):
                for mt in mts:
                    msl = slice(mt * 128, (mt + 1) * 128)
                    ps = opart.pop(mt)
                    t_o = obp.tile([128, C], MMD, tag="ob", name=f"obw{mt}")
                    nc.tensor.matmul(
                        ps[:], t_yt[KO - 1][:, msl],
                        wpb["wp"][:, KO - 1, 0:512], start=False, stop=True,
                    )
                    yield
                    nc.vector.tensor_copy(t_o[:, 0:512], ps[:])
                    nc.sync.dma_start(
                        out=r_out[mt, :, 0:512], in_=t_o[:, 0:512]
                    )
                    psB = mm_bank()
                    for tt in range(KO):
                        nc.tensor.matmul(
                            psB[:, 0:256], t_yt[tt][:, msl],
                            wpb["wp"][:, tt, 512:C],
                            start=(tt == 0), stop=(tt == KO - 1),
                        )
                        if tt < KO - 1:
                            yield
                    nc.vector.tensor_copy(t_o[:, 512:C], psB[:, 0:256])
                    nc.sync.dma_start(
                        out=r_out[mt, :, 512:C], in_=t_o[:, 512:C]
                    )
                    yield

            # ---- filler queue: generators of single-matmul steps ----
            filler_q = deque()   # (t_needed_by, generator)
            filler_q.append((1, g0))

            def fill(n):
                did = 0
                while n > 0 and filler_q:
                    try:
                        next(filler_q[0][1])
                        n -= 1
                        did += 1
                    except StopIteration:
                        filler_q.popleft()
                return did

            def drain_through(t_limit):
                keep = []
                while filler_q:
                    tnb, g = filler_q.popleft()
                    if tnb <= t_limit:
                        for _ in g:
                            pass
                    else:
                        keep.append((tnb, g))
                filler_q.extend(keep)

            # ---- the fused score/exp/AV pipeline ----
            t_yt = [None] * KO
            ya = {}      # (t, ch) -> (ya0, ya1)
            qk = {0: (q0, k0)}
            pipe = deque()  # (t, ch, mt, p_tile), AV runs two iters behind

            def emit_av(s):
                pt, pch, pmt, pp_ = s
                nsl = slice(pch * 512, (pch + 1) * 512)
                if pmt == 0:
                    ya[(pt, pch)] = (
                        yap.tile([D1, 512], F32, tag="ya", name=f"ya{pt}{pch}0"),
                        yap.tile([D1, 512], F32, tag="ya", name=f"ya{pt}{pch}1"),
                    )
                ya0, ya1 = ya[(pt, pch)]
                nc.tensor.matmul(
                    ya0[:], t_v[pmt][:, (2 * pt) * D1:(2 * pt + 1) * D1],
                    pp_[:, 0, :], start=(pmt == 0), stop=(pmt == MT - 1),
                )
                nc.tensor.matmul(
                    ya1[:], t_v[pmt][:, (2 * pt + 1) * D1:(2 * pt + 2) * D1],
                    pp_[:, 1, :], start=(pmt == 0), stop=(pmt == MT - 1),
                )
                if pmt == MT - 1:
                    # evict unnormalized yt; reciprocal of each colsum row
                    # straight off PSUM into a partition-0 staging row
                    # (partition_broadcast needs partition-0 input and
                    # offset-0 full-height output on HW)
                    # reciprocal_approx_fast needs SBUF input at partition
                    # offset 0 on HW: stage both colsum rows first, then one
                    # batched approx-recip over [1, 2, 512]
                    t_ss = csp.tile([1, 2, 512], F32, tag="ss",
                                    name=f"ss{pt}{pch}")
                    t_rs = csp.tile([1, 2, 512], F32, tag="rs",
                                    name=f"rs{pt}{pch}")
                    nc.vector.tensor_copy(t_yt[pt][0:64, nsl], ya0[0:D, :])
                    nc.vector.tensor_copy(t_ss[0:1, 0, :], ya0[D:D1, :])
                    nc.vector.tensor_copy(t_yt[pt][64:128, nsl], ya1[0:D, :])
                    nc.vector.tensor_copy(t_ss[0:1, 1, :], ya1[D:D1, :])
                    nc.vector.reciprocal_approx_fast(t_rs[0:1], t_ss[0:1])
                    del ya[(pt, pch)]
                    for hp in range(2):
                        psl = slice(hp * 64, hp * 64 + 64)
                        t_bc = bcp.tile([128, 512], F32, tag="bc",
                                        name=f"bc{pt}{pch}{hp}")
                        nc.gpsimd.partition_broadcast(
                            t_bc[:], t_rs[0:1, hp, :]
                        )
                        nc.vector.tensor_mul(
                            t_yt[pt][psl, nsl], t_yt[pt][psl, nsl],
                            t_bc[psl, :]
                        )

            for t in range(KO):
                t_yt[t] = ytp.tile([128, N], MMD, tag="yt", name=f"yt{t}")
                if t + 1 < KO:
                    wb[t + 1] = qk_dma(t + 1)
                    tq1, tk1 = qk_tiles(t + 1)
                    qk[t + 1] = (tq1, tk1)
                    filler_q.append((t + 1, qk_unit_steps(t + 1, tq1, tk1)))
                if t == 1:
                    # V' half 1 (pairs 3-5): paced behind qk(2), due by t=3
                    filler_q.append((3, vhalf1_gen()))
                drain_through(t)  # safety: pair t fully projected
                if t == 1:
                    t_wp = xw.tile([128, KO, C], MMD, tag="wpf")
                    for ko in range(KO):
                        nc.sync.dma_start(out=t_wp[:, ko, :], in_=r_wp[:, ko, :])
                    wpb["wp"] = t_wp
                t_q, t_k = qk[t]
                for ch in range(NCH):
                    nsl = slice(ch * 512, (ch + 1) * 512)
                    for mt in range(MT):
                        msl = slice(mt * 128, (mt + 1) * 128)
                        st2 = stp.tile([128, 2, 512], F32, tag="st",
                                       name=f"st{t}{ch}{mt}")
                        nc.tensor.matmul(
                            st2[:, 0, :], t_k[0:64, msl], t_q[0:64, nsl],
                            start=True, stop=True, tile_position=(0, 0),
                        )
                        nc.tensor.matmul(
                            st2[:, 1, :], t_k[64:128, msl], t_q[64:128, nsl],
                            start=True, stop=True, tile_position=(64, 0),
                        )
                        p = ppp.tile([128, 2, 512], MMD, tag="p",
                                     name=f"p{t}{ch}{mt}")
                        nc.scalar.activation(p[:], st2[:], AF.Exp)
                        vg = None
                        if t == 0 and ch == 0 and 1 <= mt < 7:
                            # weave V' half-0 [2..7] one iteration ahead of
                            # need; the group tail lands after the AV pair
                            # so its eviction wait stays off the critical
                            # path
                            vg = vprime_steps(mt + 1, 0)
                            next(vg)
                            fill(2)   # q0's ch1 half
                        elif t == 0 and ch == 0:
                            fill(2)
                        elif t == 0:
                            fill(3)   # QK(1) units: 24 steps over 8 iters
                        else:
                            if t == KO - 1 and ch == 0 and mt == 0:
                                filler_q.append(
                                    (KO, outproj_partial(range(0, 2)))
                                )
                            if t == KO - 1 and ch == 1 and mt == 1:
                                # Yt chunk 0 of every pair is normalized by
                                # now: weave the first half of the output
                                # projection into the final chunk's stream
                                filler_q.append(
                                    (KO, outproj_finish(range(0, 2)))
                                )
                                filler_q.append(
                                    (KO, outproj_steps(range(2, MT // 2)))
                                )
                            quota = 7 if (t == KO - 1 and ch == 1) else 2
                            fill(quota)
                        if len(pipe) >= 1:
                            emit_av(pipe.popleft())
                        pipe.append((t, ch, mt, p))
                        if vg is not None:
                            for _ in vg:
                                pass

            while pipe:  # flush the final AV pairs + eviction + norm
                emit_av(pipe.popleft())
            drain_through(KO)  # finish any leftover out-proj weave steps

            # tail: the 512-col groups for mt 4..7 accumulate pairs 0..4
            # (already normalized) while pair 5's normalization chain runs
            # on DVE/GpSimd, so the PE never idles on the final flush
            tailA = []
            for i, mt in enumerate(range(MT // 2, MT)):
                msl = slice(mt * 128, (mt + 1) * 128)
                ps = (st_bank if i % 2 == 0 else mm_bank)()
                for tt in range(KO - 1):
                    nc.tensor.matmul(
                        ps[:], t_yt[tt][:, msl], wpb["wp"][:, tt, 0:512],
                        start=(tt == 0), stop=False,
                    )
                tailA.append(ps)
            for i, mt in enumerate(range(MT // 2, MT)):
                msl = slice(mt * 128, (mt + 1) * 128)
                t_o = obp.tile([128, C], MMD, tag="ob", name=f"obt{mt}")
                nc.tensor.matmul(
                    tailA[i], t_yt[KO - 1][:, msl],
                    wpb["wp"][:, KO - 1, 0:512], start=False, stop=True,
                )
                nc.vector.tensor_copy(t_o[:, 0:512], tailA[i])
                nc.sync.dma_start(out=r_out[mt, :, 0:512], in_=t_o[:, 0:512])
                psB = (mm_bank if i % 2 == 0 else st_bank)()
                for tt in range(KO):
                    nc.tensor.matmul(
                        psB[:, 0:256], t_yt[tt][:, msl],
                        wpb["wp"][:, tt, 512:C],
                        start=(tt == 0), stop=(tt == KO - 1),
                    )
                nc.vector.tensor_copy(t_o[:, 512:C], psB[:, 0:256])
                nc.sync.dma_start(out=r_out[mt, :, 512:C], in_=t_o[:, 512:C])

    nc.compile()
    return nc


def _prep_inputs(x, head_mask, q_w, k_w, v_w, proj_w):
    import ml_dtypes

    mmnp = {"bf16": ml_dtypes.bfloat16, "f16": np.float16,
            "f32r": np.float32, "f32": np.float32}[MM_DTYPE]
    def pack_blocks(wT):
        # [C_in, C_out] -> [t, p, ko*128+j] with wT[ko*128+p, t*128+j]
        return np.ascontiguousarray(
            wT.reshape(KO, 128, KO, 128).transpose(2, 1, 0, 3).reshape(KO, 128, C)
        )

    scale = np.float32(D ** -0.5)
    wqT = pack_blocks((q_w.T * scale).astype(np.float32)).astype(mmnp)
    wkT = pack_blocks(k_w.T.astype(np.float32)).astype(mmnp)
    vwT0 = np.zeros((C, CV), np.float32)
    vT = v_w.T.astype(np.float32)
    for h in range(H):
        vwT0[:, h * D1:h * D1 + D] = vT[:, h * D:(h + 1) * D]
    pwT = np.ascontiguousarray(proj_w.T).astype(mmnp)
    in_maps = []
    for b in range(NCORES):
        xT = np.ascontiguousarray(x[b].T).astype(mmnp)
        # fold head_mask^2 into this core's V weights (ones cols stay 0->1)
        vwT = vwT0.copy()
        for h in range(H):
            vwT[:, h * D1:h * D1 + D] *= head_mask[b, h] ** 2
        in_maps.append(
            {"xT": xT, "wqT": wqT, "wkT": wkT, "vwT": vwT.astype(mmnp),
             "pwT": pwT}
        )
    return in_maps


def _run(inputs, trace=False):
    from concourse.bass_utils import run_bass_kernel_spmd

    x = np.asarray(inputs["x"], np.float32)
    head_mask = np.asarray(inputs["head_mask"], np.float32)
    in_maps = _prep_inputs(
        x,
        head_mask,
        np.asarray(inputs["q_w"], np.float32),
        np.asarray(inputs["k_w"], np.float32),
        np.asarray(inputs["v_w"], np.float32),
        np.asarray(inputs["proj_w"], np.float32),
    )
    # biases are zero by construction of this problem (spec fill=zeros);
    # q_b/k_b/v_b/proj_b are validated and otherwise unused.
    for name in ("q_b", "k_b", "v_b", "proj_b"):
        bias = np.asarray(inputs[name])
        if np.abs(bias).max() > 0:
            raise NotImplementedError(f"nonzero {name} not supported")

    if "nc" not in _cache:
        _cache["nc"] = _build()
    nc = _cache["nc"]
    res = run_bass_kernel_spmd(
        nc, in_maps, core_ids=list(range(NCORES)), trace=trace
    )
    out = np.stack([res.results[b]["out"] for b in range(NCORES)], axis=0)
    return out.astype(np.float32), res


def kernel(**inputs):
    out, _ = _run(inputs, trace=False)
    return out
